# revision 35
# baseline (speedup 1.0000x reference)
"""Trainium2 Bass kernel for DiffDimDotProductAttention.

reference computation:
    q = queries @ W_q                      [B, LQ, DKEY]
    scores = q @ keys^T / sqrt(DKEY)       [B, LQ, LK]
    scores = where(arange(LK) < valid_len, scores, -1e6)
    attn = softmax(scores, axis=-1)
    out = attn @ values                    [B, LQ, DV]

Sharding: LQ is split into 8 slices of 256 rows, one per NeuronCore. Every
core processes its query slice for ALL batches, so the per-core work is
identical regardless of the (runtime) valid_lens values, and key blocks that
are entirely masked (block index >= ceil(valid_len/128)) are skipped --
their attention weights are exactly zero in the reference as well.

Device layout avoids all on-chip transposes by computing scores transposed:
    qT[k, l]     = matmul(lhsT=W_q[kc_in, kc_out],  rhs=queriesT[kc_in, l])
    scoresT[s,l] = matmul(lhsT=keysT[kc, s-block],  rhs=qT[kc, l])
    P = exp(scoresT/32 + mask_col)          (masked lanes underflow to 0.0)
    out[l, v]    = matmul(lhsT=P[:, t-block], rhs=values[s-block, v])
    rowsum[l]    = matmul(lhsT=P[:, t-block], rhs=ones)
    out /= rowsum
Softmax skips the max-subtraction: scores ~ N(0,1), exp() cannot overflow,
and softmax is shift invariant, so the result matches to fp32 rounding.

The q projection runs on batch PAIRS (two 256-query slices side by side) so
its matmuls stream 512 columns and pay half the LDWEIGHTS. Batches are
processed largest-active-length first: heavy batches run while DMA prefetch
has the most runway, and the smallest batch drains the pipeline at the end.

Compute dtype "f16" stores q/K/V/W_q as float16 (10-bit mantissa, inputs are
~N(0,1) so range is irrelevant) -- half the HBM traffic of fp32 at full PE
rate, ~5e-4 relative error. "f32r" keeps fp32 storage with the PE's reduced
single-pass mode (~2.5e-4). "f32" is exact (~1e-5) but 4x slower on the PE.
"""

import math
import os
import sys

import numpy as np

DTYPE = os.environ.get("KERNEL_DTYPE", "f16")


def _ensure_paths():
    try:
        import concourse  # noqa: F401
        return
    except ImportError:
        pass
    for p in (
        "/root/.axon_site",
        "/root/.axon_site/_ro/trn_rl_repo",
        "/root/.axon_site/_ro/pypackages",
        "/opt/trn_rl_repo",
    ):
        if p not in sys.path:
            sys.path.append(p)
    import concourse  # noqa: F401


B, LQ, LK, DQ, DKEY, DV = 8, 2048, 2048, 1024, 1024, 1024
NCORES = 8
SLICE = LQ // NCORES          # 256 query rows per core
PB = 128                      # partition block
NKC = DQ // PB                # 8 contraction blocks for the projections
NSB = LK // PB                # 16 key blocks
NT = SLICE // PB              # 2 query tiles per core slice
NDV = DV // 512               # 2 value chunks of 512
KG = 8                        # key blocks per keysT SBUF tile
SCALE = 1.0 / math.sqrt(DKEY)
MASK_NEG = -1.0e6

_program_cache = {}
_last_results = None


def _np_store_dtype(dtype):
    if dtype == "bf16":
        import ml_dtypes
        return ml_dtypes.bfloat16
    if dtype == "f16":
        return np.float16
    return np.float32


def _batch_order(nb):
    """Largest batches first; qproj runs on consecutive pairs of this order."""
    order = sorted(range(B), key=lambda i: (-nb[i], i))
    pairs = [(order[2 * i], order[2 * i + 1]) for i in range(B // 2)]
    return order, pairs


def _build_program(nb, dtype):
    """Build the SPMD Bass program for per-batch active block counts `nb`."""
    _ensure_paths()
    import concourse.mybir as mybir
    import concourse.tile as tile
    from concourse import bacc

    f32 = mybir.dt.float32
    if dtype == "bf16":
        dt_s = mybir.dt.bfloat16        # storage/compute dtype
    elif dtype == "f16":
        dt_s = mybir.dt.float16
    elif dtype == "f32r":
        dt_s = mybir.dt.float32r
    else:
        dt_s = f32
    Exp = mybir.ActivationFunctionType.Exp

    nc = bacc.Bacc("TRN2", target_bir_lowering=False, debug=False,
                   num_devices=NCORES)
    # qT holds the transposed query slices of a batch PAIR side by side
    qT = nc.declare_dram_parameter("qT", [B // 2, PB, NKC * 2 * SLICE], dt_s,
                                   isOutput=False)
    kT = nc.declare_dram_parameter("kT", [B, NKC, PB, LK], dt_s, isOutput=False)
    vv = nc.declare_dram_parameter("vv", [B, NSB, PB, DV], dt_s, isOutput=False)
    wq = nc.declare_dram_parameter("wq", [NKC, PB, DKEY], dt_s, isOutput=False)
    mk = nc.declare_dram_parameter("mk", [B, PB, NSB], f32, isOutput=False)
    out = nc.declare_dram_parameter("out", [B, SLICE, DV], f32, isOutput=True)

    MM = nc.tensor.matmul
    _, pairs = _batch_order(nb)

    with tile.TileContext(nc) as tc, \
         tc.tile_pool(name="const", bufs=1) as constp, \
         tc.tile_pool(name="qtr", bufs=2) as qtrp, \
         tc.tile_pool(name="qt", bufs=2) as qtp, \
         tc.tile_pool(name="kt", bufs=3) as ktp, \
         tc.tile_pool(name="vt", bufs=6) as vtp, \
         tc.tile_pool(name="pt", bufs=4) as ptp, \
         tc.tile_pool(name="mask", bufs=2) as maskp, \
         tc.tile_pool(name="outs", bufs=3) as outsp, \
         tc.tile_pool(name="rcp", bufs=4) as rcpp, \
         tc.tile_pool(name="ps_a", bufs=2, space="PSUM") as psa, \
         tc.tile_pool(name="ps_o", bufs=4, space="PSUM") as pso, \
         tc.tile_pool(name="ps_r", bufs=2, space="PSUM") as psr:

        wq_sb = constp.tile([PB, NKC, DKEY], dt_s)
        qtr0_sb = qtrp.tile([PB, NKC * 2 * SLICE], dt_s, tag="qtr", name="qtr0")
        # interleave W_q and first-pair query blocks across both trigger queues
        # so the first qproj group can start streaming as soon as block 0 lands
        for kc in range(NKC):
            nc.sync.dma_start(out=wq_sb[:, kc, :], in_=wq[kc])
            nc.gpsimd.dma_start(
                out=qtr0_sb[:, kc * 2 * SLICE:(kc + 1) * 2 * SLICE],
                in_=qT[0, :, kc * 2 * SLICE:(kc + 1) * 2 * SLICE])
        ones_sb = constp.tile([PB, 1], f32 if dtype == "f32r" else dt_s)
        nc.vector.memset(ones_sb, 1.0)

        # HAM warm-up: ~5us of throwaway matmuls on zeroed SBUF while the
        # first W_q/queries DMA wave lands, so real work starts at 2.4 GHz
        warm_sb = constp.tile([PB, 512], dt_s, name="warm_sb")
        nc.vector.memset(warm_sb, 0.0)
        wps = psa.tile([PB, 512], f32, tag="ps_a", name="warmps")
        for i in range(24):
            MM(wps, warm_sb[:, :PB], warm_sb, start=(i == 0), stop=(i == 23))

        def qproj(bp, qtr_sb):
            """Project a batch pair: qt[:, kco*512 + bi*256 + l]."""
            qt_sb = qtp.tile([PB, NKC * 2 * SLICE], dt_s, tag="qt",
                             name=f"qtp{bp}")
            for kco in range(NKC):
                ps = psa.tile([PB, 2 * SLICE], f32, tag="ps_a", name="psq")
                for kci in range(NKC):
                    MM(ps,
                       wq_sb[:, kci, kco * PB:(kco + 1) * PB],
                       qtr_sb[:, kci * 2 * SLICE:(kci + 1) * 2 * SLICE],
                       start=(kci == 0), stop=(kci == NKC - 1))
                nc.vector.tensor_copy(
                    qt_sb[:, kco * 2 * SLICE:(kco + 1) * 2 * SLICE], ps)
            return qt_sb

        def attention(b, qt_sb, bi, first_batch=False):
            nbb = nb[b]

            mask_sb = maskp.tile([PB, NSB], f32, tag="mask", name="mask")
            nc.sync.dma_start(out=mask_sb, in_=mk[b])

            ops = [pso.tile([PB, 512], f32, tag="ps_o", name=f"ops{i}")
                   for i in range(NT * NDV)]
            rs = [psr.tile([PB, 1], f32, tag="ps_r", name=f"rs{i}")
                  for i in range(NT)]

            def attn_v(s, pt_sb, v_sb):
                first, last = s == 0, s == nbb - 1
                for t in range(NT):
                    ptt = pt_sb[:, t * PB:(t + 1) * PB]
                    for dv in range(NDV):
                        MM(ops[t * NDV + dv], ptt,
                           v_sb[:, dv * 512:(dv + 1) * 512],
                           start=first, stop=last)
                    # fp32r bits are plain f32 with a zeroed tail; run the tiny
                    # N=1 rowsum matmul in f32 (fp32r rejects this dst shape)
                    rsl = ptt.bitcast(f32) if dtype == "f32r" else ptt
                    nc.tensor.matmul(rs[t], rsl, ones_sb, start=first, stop=last)

            kt_tiles = {}
            pending = None  # software-pipeline attn_v by one block
            for s in range(nbb):
                g, so_k = divmod(s, KG)
                gw = min(KG, nbb - g * KG)  # blocks in this key group
                if so_k == 0:
                    ktg = ktp.tile([PB, NKC * gw * PB], dt_s, tag="kt",
                                   name=f"ktg{g}")
                    for kc in range(NKC):
                        # first group of the first batch races the W_q wave;
                        # split it across both trigger queues
                        split = first_batch and g == 0 and kc % 2
                        eng = nc.gpsimd if split else nc.sync
                        eng.dma_start(
                            out=ktg[:, kc * gw * PB:(kc + 1) * gw * PB],
                            in_=kT[b, kc, :, g * KG * PB:g * KG * PB + gw * PB])
                    kt_tiles[g] = ktg
                v_sb = vtp.tile([PB, DV], dt_s, tag="vt", name="vt")
                nc.gpsimd.dma_start(out=v_sb, in_=vv[b, s])

                ktg = kt_tiles[g]
                ps = psa.tile([PB, SLICE], f32, tag="ps_a", name="pss")
                qbase = bi * SLICE
                for kc in range(NKC):
                    base = kc * gw * PB + so_k * PB
                    MM(ps, ktg[:, base:base + PB],
                       qt_sb[:, kc * 2 * SLICE + qbase:
                             kc * 2 * SLICE + qbase + SLICE],
                       start=(kc == 0), stop=(kc == NKC - 1))
                pt_sb = ptp.tile([PB, SLICE], dt_s, tag="pt", name="pt")
                nc.scalar.activation(pt_sb, ps, Exp,
                                     bias=mask_sb[:, s:s + 1], scale=SCALE)
                if pending is not None:
                    attn_v(*pending)
                pending = (s, pt_sb, v_sb)
            attn_v(*pending)

            # ---- normalize and store
            for t in range(NT):
                rcp = rcpp.tile([PB, 1], f32, tag="rcp", name="rcp")
                nc.vector.reciprocal(rcp, rs[t])
                out_sb = outsp.tile([PB, DV], f32, tag="outs", name="outs")
                for dv in range(NDV):
                    nc.vector.tensor_scalar_mul(
                        out_sb[:, dv * 512:(dv + 1) * 512],
                        ops[t * NDV + dv], rcp)
                nc.sync.dma_start(out=out[b, t * PB:(t + 1) * PB, :], in_=out_sb)

        for bp, (b0, b1) in enumerate(pairs):
            if bp == 0:
                qtr_sb = qtr0_sb
            else:
                qtr_sb = qtrp.tile([PB, NKC * 2 * SLICE], dt_s, tag="qtr",
                                   name="qtr")
                nc.sync.dma_start(out=qtr_sb, in_=qT[bp])
            qt_sb = qproj(bp, qtr_sb)
            attention(b0, qt_sb, 0, first_batch=(bp == 0))
            attention(b1, qt_sb, 1)

    nc.compile()
    return nc


def _round_fp32r(a):
    """Round float32 array to the fp32r grid (1+8+11 bits, RNE)."""
    u = np.ascontiguousarray(a, dtype=np.float32).view(np.uint32)
    lsb = (u >> np.uint32(12)) & np.uint32(1)
    u = u + np.uint32(0x7FF) + lsb
    u &= np.uint32(0xFFFFF000)
    return u.view(np.float32)


def _prepare(inputs, dtype):
    np_s = _np_store_dtype(dtype)
    queries = np.ascontiguousarray(np.asarray(inputs["queries"], dtype=np.float32))
    keys = np.asarray(inputs["keys"], dtype=np.float32)
    values = np.asarray(inputs["values"], dtype=np.float32)
    valid_lens = np.asarray(inputs["valid_lens"]).astype(np.int64)
    W_q = np.asarray(inputs["W_q"], dtype=np.float32)

    nb = tuple(int(min(NSB, max(1, math.ceil(int(v) / PB)))) for v in valid_lens)
    _, pairs = _batch_order(nb)

    if dtype == "f32r":
        queries = _round_fp32r(queries)
        keys = _round_fp32r(keys)
        values = _round_fp32r(values)
        W_q = _round_fp32r(W_q)

    # kT[b, kc, p, k] = keys[b, k, kc*128+p]  (key index contiguous)
    kT = np.ascontiguousarray(
        keys.reshape(B, LK, NKC, PB).transpose(0, 2, 3, 1).astype(np_s))
    # vv[b, s, p, v] = values[b, s*128+p, v]  (natural blocks)
    vvb = np.ascontiguousarray(values.reshape(B, NSB, PB, DV).astype(np_s))
    # W_q blocked by contraction rows, contiguous
    wqb = np.ascontiguousarray(W_q.reshape(NKC, PB, DKEY).astype(np_s))
    # additive mask columns: mk[b, p, s] = 0 if s*128+p < valid_len else -1e6
    pos = (np.arange(NSB)[None, :] * PB + np.arange(PB)[:, None])  # [PB, NSB]
    mkb = np.where(pos[None, :, :] < valid_lens[:, None, None], 0.0,
                   MASK_NEG).astype(np.float32)
    mkb = np.ascontiguousarray(mkb)

    # per-core transposed query slices, batch-pair interleaved:
    # qT[bp, p, kc*512 + bi*256 + l] = queries[pairs[bp][bi], base+l, kc*128+p]
    in_maps = []
    for c in range(NCORES):
        qs = queries[:, c * SLICE:(c + 1) * SLICE, :]  # [B, SLICE, DQ]
        qsT = qs.reshape(B, SLICE, NKC, PB).transpose(0, 2, 3, 1)  # [B,kc,p,l]
        qTb = np.empty((B // 2, PB, NKC * 2 * SLICE), dtype=np_s)
        for bp, (b0, b1) in enumerate(pairs):
            blk = qTb[bp].reshape(PB, NKC, 2, SLICE)
            blk[:, :, 0, :] = qsT[b0].transpose(1, 0, 2).astype(np_s)
            blk[:, :, 1, :] = qsT[b1].transpose(1, 0, 2).astype(np_s)
        in_maps.append({"qT": qTb, "kT": kT, "vv": vvb, "wq": wqb, "mk": mkb})
    return nb, in_maps


def _run(inputs, trace=False, dtype=None):
    _ensure_paths()
    from concourse.bass_utils import run_bass_kernel_spmd

    dtype = dtype or DTYPE
    nb, in_maps = _prepare(inputs, dtype)
    key = (nb, dtype)
    if key not in _program_cache:
        _program_cache[key] = _build_program(nb, dtype)
    nc = _program_cache[key]

    core_ids = list(range(NCORES))
    res = run_bass_kernel_spmd(nc, in_maps, core_ids, trace=trace)
    global _last_results
    _last_results = res
    full = np.concatenate([res.results[c]["out"] for c in range(NCORES)], axis=1)
    return full, res.exec_time_ns


def kernel(**inputs) -> np.ndarray:
    return _run(inputs, trace=False)[0]
